# revision 20
# baseline (speedup 1.0000x reference)
"""Chunked-prefill paged attention kernel for Trainium2 (Bass/Tile), 8 cores.

Sharding: tensor-parallel over heads. Core i handles q heads 4i..4i+3 and
kv head i. The paged-cache scatter/gather (index-driven data movement) is
resolved on the host; each core runs dense attention over the gathered
[ctx | chunk] keys/values for its kv head.

Per-core structure ("transposed scores"): loop over (q-chunk c, head-pair
hp); inner loop over 128-row l-tiles, software-pipelined one step so the
activation engine (the bottleneck at ~1.15 us per [128,2,512] exp) never
starves:
  - 2 QK^T matmuls (fp16, kv-head kT stationary shared by both heads,
    LDWEIGHTS fully hidden behind the streams) -> fp32 PSUM pair tile
    [128,2,512] (2 banks, double-buffered).
  - causal mask: DVE adds a NEG-triangle on the diagonal 128-block; QK/PV
    and the exp are exactly trimmed to the visible q-columns.
  - ONE activation exps both heads' scores -> fp16 ex tile in SBUF.
  - 2 PV matmuls (fp16) accumulate into per-head PSUM banks.
  - 2 col-tiled (tile_position) ones-matmuls run CONCURRENTLY on separate
    XBUSes, accumulating both heads' softmax denominators into rows
    {0,32} of ONE persistent PSUM bank across the whole pass - one
    512-col stream per tile instead of two.
PSUM: 4 (score pairs x2) + 2 (accumulators) + 1 (denominators) = 7 banks.
The unnormalized oT and denominators are DMA'd out; the host does the
final divide and [d, q] -> [q, d] transpose.
"""

import numpy as np

import concourse.bacc as bacc
import concourse.bass as bass
import concourse.mybir as mybir
import concourse.tile as tile
from concourse.bass_utils import run_bass_kernel_spmd

NH, NKVH, HD = 32, 8, 128
SCALE = 0.08838834764831845  # 1/sqrt(128)
SEQ, CTX = 1024, 3072
L = CTX + SEQ  # 4096
NDEV = 8
HPD = NH // NDEV  # q heads per device
QCH = 512  # q columns per chunk (psum bank width in f32)
NQC = SEQ // QCH
NT = L // 128  # 32 l-tiles
NT_CTX = CTX // 128  # 24 context l-tiles
NEG = -1.0e30

F32 = mybir.dt.float32
FP16 = mybir.dt.float16

_CACHE = {}


def _tiles_for_chunk(c):
    """(lt, st, diag) per l-tile: st = first visible q-col, diag = needs
    triangular mask at cols [st, st+128)."""
    out = [(lt, 0, False) for lt in range(NT_CTX)]
    for b in range(4 * (c + 1)):
        st = 128 * b - QCH * c
        out.append((NT_CTX + b, max(st, 0), st >= 0))
    return out


def _build():
    nc = bacc.Bacc("TRN2", target_bir_lowering=False, debug=False)

    NKC = 4
    TPC = NT // NKC  # l-tiles per load chunk
    qdT = nc.dram_tensor("qdT", [HPD * HD, SEQ], FP16, kind="ExternalInput")
    kdT = nc.dram_tensor(
        "kdT", [NKC, HD, L // NKC], FP16, kind="ExternalInput"
    )
    vd = nc.dram_tensor(
        "vd", [NKC, HD, TPC, HD], FP16, kind="ExternalInput"
    )
    tri = nc.dram_tensor("tri", [128, 128], FP16, kind="ExternalInput")
    od = nc.dram_tensor(
        "od", [NQC, HPD, HD, QCH], FP16, kind="ExternalOutput"
    )
    sums_out = nc.dram_tensor(
        "sums", [NQC * HPD // 2, 4, QCH], F32, kind="ExternalOutput"
    )

    with tile.TileContext(nc) as tc:
        with (
            tc.tile_pool(name="inp", bufs=1) as inp,
            tc.tile_pool(name="small", bufs=1) as small,
            tc.tile_pool(name="exq", bufs=10) as exq,
            tc.tile_pool(name="ssb", bufs=2) as ssb,
            tc.tile_pool(name="osb", bufs=4) as osb,
            tc.tile_pool(name="scp", bufs=2, space="PSUM") as scp,
            tc.tile_pool(name="accps", bufs=1, space="PSUM") as accps,
            tc.tile_pool(name="sumps", bufs=1, space="PSUM") as sumps,
        ):
            # ---- constants ----
            tri_sb = small.tile([128, 128], FP16, tag="tri")
            nc.scalar.dma_start(out=tri_sb, in_=tri[:, :])
            ones_f = small.tile([128, 1], F32, tag="ones_f")
            nc.vector.memset(ones_f, 1.0)
            ones_h = small.tile([128, 1], FP16, tag="ones")
            nc.vector.tensor_copy(out=ones_h, in_=ones_f)

            # ---- input loads: all-contiguous fp16 DMA chunks.
            # DMA triggers cost ~650ns on the issuing engine: keep few. ----
            kT = [
                inp.tile([128, L // NKC], FP16, name=f"kT{i}", tag=f"kT{i}")
                for i in range(NKC)
            ]
            qT = [
                inp.tile([128, SEQ], FP16, name=f"qT{h}", tag=f"qT{h}")
                for h in range(HPD)
            ]
            v_h = [
                inp.tile([128, TPC, HD], FP16, name=f"v{i}", tag=f"v{i}")
                for i in range(NKC)
            ]

            def load_k(i):
                nc.sync.dma_start(out=kT[i], in_=kdT[i, :, :])

            def load_v(i):
                nc.scalar.dma_start(out=v_h[i], in_=vd[i, :, :, :])

            load_k(0)
            nc.sync.dma_start(out=qT[0], in_=qdT[0:128, :])
            nc.sync.dma_start(out=qT[1], in_=qdT[128:256, :])
            load_v(0)
            for i in range(1, NKC):
                load_k(i)
                load_v(i)
            for h in range(2, HPD):
                nc.sync.dma_start(
                    out=qT[h], in_=qdT[h * 128 : (h + 1) * 128, :]
                )

            def kT_at(lt):
                return kT[lt // TPC][
                    :, (lt % TPC) * 128 : (lt % TPC + 1) * 128
                ]

            def v_at(lt):
                return v_h[lt // TPC][:, lt % TPC, :]

            assert TPC == 8  # kT_at/v_at index by lt // TPC

            # ---- main: 4 passes (q-chunk c x head-pair hp) ----
            for c in range(NQC):
                tiles = _tiles_for_chunk(c)
                last_i = len(tiles) - 1
                for hp in range(HPD // 2):
                    h0 = 2 * hp
                    acc = [
                        accps.tile([128, QCH], F32, name=f"acc{j}", tag=f"acc{j}")
                        for j in range(2)
                    ]
                    sums_ps = sumps.tile(
                        [97, QCH], F32, name="sums_ps", tag="sums_ps"
                    )
                    ex_tiles = [None] * len(tiles)

                    def emit_qk(i, lt, st, diag):
                        qsl = slice(c * QCH + st, (c + 1) * QCH)
                        pair = scp.tile(
                            [128, 2, QCH], F32, name="pair", tag="pair"
                        )
                        for j in range(2):
                            nc.tensor.matmul(
                                pair[:, j, st:],
                                kT_at(lt),
                                qT[h0 + j][:, qsl],
                                start=True,
                                stop=True,
                            )
                        exi = exq.tile(
                            [128, 2, QCH], FP16, name="exi", tag="ex"
                        )
                        nc.scalar.activation(
                            out=exi[:, :, st:],
                            in_=pair[:, :, st:],
                            func=mybir.ActivationFunctionType.Exp,
                            scale=SCALE,
                        )
                        if diag:
                            for j in range(2):
                                nc.vector.tensor_mul(
                                    out=exi[:, j, st : st + 128],
                                    in0=exi[:, j, st : st + 128],
                                    in1=tri_sb,
                                )
                        ex_tiles[i] = exi

                    def emit_sums(i0):
                        # 8 denominator matmuls for 4 iterations in two
                        # 4-way col-tiled concurrent waves; row =
                        # 64*(iter parity) + 32*(head), accumulated
                        # across the pass via has_written flags.
                        for i in range(i0, i0 + 4):
                            lt, st, diag = tiles[i]
                            exi = ex_tiles[i]
                            for j in range(2):
                                r = 64 * (i % 2) + 32 * j
                                nc.tensor.matmul(
                                    sums_ps[r : r + 1, st:],
                                    ones_h,
                                    exi[:, j, st:],
                                    start=(i < 2),
                                    stop=(i >= last_i - 1),
                                    tile_position=(0, r),
                                    skip_group_check=True,
                                )

                    def emit_pv(ia, ib):
                        for i in (ia, ib):
                            lt, st, diag = tiles[i]
                            exi = ex_tiles[i]
                            for j in range(2):
                                nc.tensor.matmul(
                                    acc[j][:, st:],
                                    v_at(lt),
                                    exi[:, j, st:],
                                    start=(i == 0),
                                    stop=(i == last_i),
                                    skip_group_check=True,
                                )

                    n = len(tiles)
                    for pi in range(0, n, 2):
                        emit_qk(pi, *tiles[pi])
                        emit_qk(pi + 1, *tiles[pi + 1])
                        if pi % 4 == 0 and pi >= 8:
                            emit_sums(pi - 8)
                        if pi >= 4:
                            emit_pv(pi - 4, pi - 3)
                    emit_pv(n - 4, n - 3)
                    emit_sums(n - 8)
                    emit_pv(n - 2, n - 1)
                    emit_sums(n - 4)

                    # ---- drains ----
                    pidx = c * (HPD // 2) + hp
                    sums_sb = ssb.tile([97, QCH], F32, tag="sums_sb")
                    nc.vector.tensor_copy(out=sums_sb, in_=sums_ps)
                    nc.sync.dma_start(
                        out=sums_out[pidx, :, :],
                        in_=sums_sb[0:97:32, :],
                    )
                    for j in range(2):
                        acc_sb = osb.tile([128, QCH], FP16, tag="acc_sb")
                        if j == 0:
                            nc.vector.tensor_copy(out=acc_sb, in_=acc[j])
                        else:
                            nc.scalar.copy(out=acc_sb, in_=acc[j])
                        nc.sync.dma_start(
                            out=od[c, h0 + j, :, :], in_=acc_sb
                        )
    nc.compile()
    return nc


def _prep_host(q, k, v, k_cache, v_cache, slot_mapping, context_slots):
    """Resolve the paged-cache scatter+gather on the host."""
    kh = np.ascontiguousarray(k).reshape(SEQ, NKVH, HD)
    vh = np.ascontiguousarray(v).reshape(SEQ, NKVH, HD)
    sm = np.asarray(slot_mapping)
    cs = np.asarray(context_slots)

    k_ctx = np.asarray(k_cache)[cs].copy()
    v_ctx = np.asarray(v_cache)[cs].copy()
    order = np.argsort(sm, kind="stable")
    ss = sm[order]
    j = np.searchsorted(ss, cs)
    jc = np.minimum(j, len(ss) - 1)
    hit = ss[jc] == cs
    if hit.any():
        src = order[jc[hit]]
        k_ctx[hit] = kh[src]
        v_ctx[hit] = vh[src]

    k_all = np.concatenate([k_ctx, kh], axis=0)  # [L, NKVH, HD]
    v_all = np.concatenate([v_ctx, vh], axis=0)
    return k_all, v_all


# results of the last run (exec time etc), for the local test harness
last_results = None


def kernel(q, k, v, k_cache, v_cache, slot_mapping, context_slots):
    global last_results
    q = np.asarray(q, dtype=np.float32)
    k_all, v_all = _prep_host(
        q, np.asarray(k), np.asarray(v), k_cache, v_cache,
        slot_mapping, context_slots,
    )

    if "nc" not in _CACHE:
        _CACHE["nc"] = _build()
    nc = _CACHE["nc"]

    tri = np.where(
        np.arange(128)[None, :] >= np.arange(128)[:, None], 1.0, 0.0
    ).astype(np.float16)

    in_maps = []
    for d in range(NDEV):
        in_maps.append(
            {
                "qdT": np.ascontiguousarray(
                    q[:, d * HPD * HD : (d + 1) * HPD * HD].T
                ).astype(np.float16),
                # [NKC, HD, L//NKC]: contiguous per-chunk kT blocks
                "kdT": np.ascontiguousarray(
                    k_all[:, d, :]
                    .T.reshape(HD, 4, L // 4)
                    .transpose(1, 0, 2)
                ).astype(np.float16),
                # [NKC, 128, TPC, HD]: partition p holds v[tile*128+p, :]
                "vd": np.ascontiguousarray(
                    v_all[:, d, :]
                    .reshape(4, 8, 128, HD)
                    .transpose(0, 2, 1, 3)
                ).astype(np.float16),
                "tri": tri,
            }
        )

    res = run_bass_kernel_spmd(nc, in_maps, core_ids=list(range(NDEV)))
    last_results = res

    out = np.empty((SEQ, NH * HD), dtype=np.float32)
    for d in range(NDEV):
        odr = res.results[d]["od"].astype(np.float32)
        oT = odr.transpose(1, 2, 0, 3).reshape(HPD, HD, SEQ)
        sb = res.results[d]["sums"]  # [NQC*HPD//2, 4, QCH]
        sums = np.empty((HPD, SEQ), dtype=np.float32)
        for c in range(NQC):
            for hp in range(HPD // 2):
                blk = sb[c * (HPD // 2) + hp]
                for j in range(2):
                    sums[2 * hp + j, c * QCH : (c + 1) * QCH] = (
                        blk[j] + blk[2 + j]
                    )
        o = oT / sums[:, None, :]
        out[:, d * HPD * HD : (d + 1) * HPD * HD] = (
            o.transpose(2, 0, 1).reshape(SEQ, HPD * HD)
        )
    return out


# revision 21
# speedup vs baseline: 1.0194x; 1.0194x over previous
"""Chunked-prefill paged attention kernel for Trainium2 (Bass/Tile), 8 cores.

Sharding: tensor-parallel over heads. Core i handles q heads 4i..4i+3 and
kv head i. The paged-cache scatter/gather (index-driven data movement) is
resolved on the host; each core runs dense attention over the gathered
[ctx | chunk] keys/values for its kv head.

Per-core structure ("transposed scores"): loop over (q-chunk c, head-pair
hp); inner loop over 128-row l-tiles, software-pipelined one step so the
activation engine (the bottleneck at ~1.15 us per [128,2,512] exp) never
starves:
  - 2 QK^T matmuls (fp16, kv-head kT stationary shared by both heads,
    LDWEIGHTS fully hidden behind the streams) -> fp32 PSUM pair tile
    [128,2,512] (2 banks, double-buffered).
  - causal mask: DVE adds a NEG-triangle on the diagonal 128-block; QK/PV
    and the exp are exactly trimmed to the visible q-columns.
  - ONE activation exps both heads' scores -> fp16 ex tile in SBUF.
  - 2 PV matmuls (fp16) accumulate into per-head PSUM banks.
  - 2 col-tiled (tile_position) ones-matmuls run CONCURRENTLY on separate
    XBUSes, accumulating both heads' softmax denominators into rows
    {0,32} of ONE persistent PSUM bank across the whole pass - one
    512-col stream per tile instead of two.
PSUM: 4 (score pairs x2) + 2 (accumulators) + 1 (denominators) = 7 banks.
The unnormalized oT and denominators are DMA'd out; the host does the
final divide and [d, q] -> [q, d] transpose.
"""

import numpy as np

import concourse.bacc as bacc
import concourse.bass as bass
import concourse.mybir as mybir
import concourse.tile as tile
from concourse.bass_utils import run_bass_kernel_spmd

NH, NKVH, HD = 32, 8, 128
SCALE = 0.08838834764831845  # 1/sqrt(128)
SEQ, CTX = 1024, 3072
L = CTX + SEQ  # 4096
NDEV = 8
HPD = NH // NDEV  # q heads per device
QCH = 512  # q columns per chunk (psum bank width in f32)
NQC = SEQ // QCH
NT = L // 128  # 32 l-tiles
NT_CTX = CTX // 128  # 24 context l-tiles
NEG = -1.0e30

F32 = mybir.dt.float32
FP16 = mybir.dt.float16

_CACHE = {}


def _tiles_for_chunk(c):
    """(lt, st, diag) per l-tile: st = first visible q-col, diag = needs
    triangular mask at cols [st, st+128)."""
    out = [(lt, 0, False) for lt in range(NT_CTX)]
    for b in range(4 * (c + 1)):
        st = 128 * b - QCH * c
        out.append((NT_CTX + b, max(st, 0), st >= 0))
    return out


def _build():
    nc = bacc.Bacc("TRN2", target_bir_lowering=False, debug=False)

    NKC = 4
    TPC = NT // NKC  # l-tiles per load chunk
    qdT = nc.dram_tensor("qdT", [HPD * HD, SEQ], FP16, kind="ExternalInput")
    kdT = nc.dram_tensor(
        "kdT", [NKC, HD, L // NKC], FP16, kind="ExternalInput"
    )
    vd = nc.dram_tensor(
        "vd", [NKC, HD, TPC, HD], FP16, kind="ExternalInput"
    )
    tri = nc.dram_tensor("tri", [128, 128], FP16, kind="ExternalInput")
    od = nc.dram_tensor(
        "od", [NQC, HPD, HD, QCH], FP16, kind="ExternalOutput"
    )
    sums_out = nc.dram_tensor(
        "sums", [NQC * HPD // 2, 4, QCH], F32, kind="ExternalOutput"
    )

    with tile.TileContext(nc) as tc:
        with (
            tc.tile_pool(name="inp", bufs=1) as inp,
            tc.tile_pool(name="small", bufs=1) as small,
            tc.tile_pool(name="exq", bufs=10) as exq,
            tc.tile_pool(name="ssb", bufs=2) as ssb,
            tc.tile_pool(name="osb", bufs=4) as osb,
            tc.tile_pool(name="scp", bufs=2, space="PSUM") as scp,
            tc.tile_pool(name="accps", bufs=1, space="PSUM") as accps,
            tc.tile_pool(name="sumps", bufs=1, space="PSUM") as sumps,
        ):
            # ---- constants ----
            tri_sb = small.tile([128, 128], FP16, tag="tri")
            nc.scalar.dma_start(out=tri_sb, in_=tri[:, :])
            ones_f = small.tile([128, 1], F32, tag="ones_f")
            nc.vector.memset(ones_f, 1.0)
            ones_h = small.tile([128, 1], FP16, tag="ones")
            nc.vector.tensor_copy(out=ones_h, in_=ones_f)

            # ---- input loads: all-contiguous fp16 DMA chunks.
            # DMA triggers cost ~650ns on the issuing engine: keep few. ----
            kT = [
                inp.tile([128, L // NKC], FP16, name=f"kT{i}", tag=f"kT{i}")
                for i in range(NKC)
            ]
            qT = [
                inp.tile([128, SEQ], FP16, name=f"qT{h}", tag=f"qT{h}")
                for h in range(HPD)
            ]
            v_h = [
                inp.tile([128, TPC, HD], FP16, name=f"v{i}", tag=f"v{i}")
                for i in range(NKC)
            ]

            def load_k(i):
                nc.sync.dma_start(out=kT[i], in_=kdT[i, :, :])

            def load_v(i):
                nc.scalar.dma_start(out=v_h[i], in_=vd[i, :, :, :])

            load_k(0)
            nc.sync.dma_start(out=qT[0], in_=qdT[0:128, :])
            nc.sync.dma_start(out=qT[1], in_=qdT[128:256, :])
            load_v(0)
            for i in range(1, NKC):
                load_k(i)
                load_v(i)
            for h in range(2, HPD):
                nc.sync.dma_start(
                    out=qT[h], in_=qdT[h * 128 : (h + 1) * 128, :]
                )

            def kT_at(lt):
                return kT[lt // TPC][
                    :, (lt % TPC) * 128 : (lt % TPC + 1) * 128
                ]

            def v_at(lt):
                return v_h[lt // TPC][:, lt % TPC, :]

            assert TPC == 8  # kT_at/v_at index by lt // TPC

            # ---- main: one flat software pipeline across all 4 passes
            # (q-chunk c x head-pair hp), so the pipeline never drains at
            # pass boundaries: pass p's PV/sums tails interleave with pass
            # p+1's QK/exp ramp. ----
            passes = [
                (c, hp, _tiles_for_chunk(c))
                for c in range(NQC)
                for hp in range(HPD // 2)
            ]
            flat = [
                (p, i)
                for p, (_, _, tiles) in enumerate(passes)
                for i in range(len(tiles))
            ]
            ctx = {}  # pass idx -> dict(acc, sums_ps, ex_tiles)

            def emit_qk(p, i):
                c, hp, tiles = passes[p]
                if i == 0:
                    ctx[p] = {
                        "acc": [
                            accps.tile(
                                [128, QCH], F32, name=f"acc{j}", tag=f"acc{j}"
                            )
                            for j in range(2)
                        ],
                        "sums_ps": sumps.tile(
                            [97, QCH], F32, name="sums_ps", tag="sums_ps"
                        ),
                        "ex": [None] * len(tiles),
                    }
                lt, st, diag = tiles[i]
                qsl = slice(c * QCH + st, (c + 1) * QCH)
                pair = scp.tile([128, 2, QCH], F32, name="pair", tag="pair")
                for j in range(2):
                    nc.tensor.matmul(
                        pair[:, j, st:],
                        kT_at(lt),
                        qT[2 * hp + j][:, qsl],
                        start=True,
                        stop=True,
                    )
                exi = exq.tile([128, 2, QCH], FP16, name="exi", tag="ex")
                nc.scalar.activation(
                    out=exi[:, :, st:],
                    in_=pair[:, :, st:],
                    func=mybir.ActivationFunctionType.Exp,
                    scale=SCALE,
                )
                if diag:
                    for j in range(2):
                        nc.vector.tensor_mul(
                            out=exi[:, j, st : st + 128],
                            in0=exi[:, j, st : st + 128],
                            in1=tri_sb,
                        )
                ctx[p]["ex"][i] = exi

            def emit_sums(p, i0):
                # 8 denominator matmuls for 4 iterations in two 4-way
                # col-tiled concurrent waves; row = 64*(iter parity) +
                # 32*(head), accumulated across the pass in one PSUM bank.
                _, _, tiles = passes[p]
                cp = ctx[p]
                last_i = len(tiles) - 1
                for i in range(i0, i0 + 4):
                    lt, st, diag = tiles[i]
                    exi = cp["ex"][i]
                    for j in range(2):
                        r = 64 * (i % 2) + 32 * j
                        nc.tensor.matmul(
                            cp["sums_ps"][r : r + 1, st:],
                            ones_h,
                            exi[:, j, st:],
                            start=(i < 2),
                            stop=(i >= last_i - 1),
                            tile_position=(0, r),
                            skip_group_check=True,
                        )
                if i0 + 4 == len(tiles):
                    emit_drains(p)

            def emit_pv(p, ia):
                _, _, tiles = passes[p]
                cp = ctx[p]
                last_i = len(tiles) - 1
                for i in (ia, ia + 1):
                    lt, st, diag = tiles[i]
                    exi = cp["ex"][i]
                    for j in range(2):
                        nc.tensor.matmul(
                            cp["acc"][j][:, st:],
                            v_at(lt),
                            exi[:, j, st:],
                            start=(i == 0),
                            stop=(i == last_i),
                            skip_group_check=True,
                        )

            def emit_drains(p):
                c, hp, tiles = passes[p]
                cp = ctx[p]
                pidx = c * (HPD // 2) + hp
                sums_sb = ssb.tile([97, QCH], F32, tag="sums_sb")
                nc.vector.tensor_copy(out=sums_sb, in_=cp["sums_ps"])
                nc.sync.dma_start(
                    out=sums_out[pidx, :, :], in_=sums_sb[0:97:32, :]
                )
                for j in range(2):
                    acc_sb = osb.tile([128, QCH], FP16, tag="acc_sb")
                    if j == 0:
                        nc.vector.tensor_copy(out=acc_sb, in_=cp["acc"][j])
                    else:
                        nc.scalar.copy(out=acc_sb, in_=cp["acc"][j])
                    nc.sync.dma_start(out=od[c, 2 * hp + j, :, :], in_=acc_sb)

            # pipeline: QK at w, PV lags 4 iterations, sums lag 8
            for w in range(0, len(flat) + 8, 2):
                if w < len(flat):
                    emit_qk(*flat[w])
                    emit_qk(*flat[w + 1])
                if w >= 8 and (w - 8) % 4 == 0 and w - 8 < len(flat):
                    emit_sums(*flat[w - 8])
                if w >= 4 and w - 4 < len(flat):
                    emit_pv(*flat[w - 4])
            # note: emit_sums at i0+4==n triggers the pass's drains; the
            # (w-8)%4 alignment holds because every pass length %4 == 0
    nc.compile()
    return nc


def _prep_host(q, k, v, k_cache, v_cache, slot_mapping, context_slots):
    """Resolve the paged-cache scatter+gather on the host."""
    kh = np.ascontiguousarray(k).reshape(SEQ, NKVH, HD)
    vh = np.ascontiguousarray(v).reshape(SEQ, NKVH, HD)
    sm = np.asarray(slot_mapping)
    cs = np.asarray(context_slots)

    k_ctx = np.asarray(k_cache)[cs].copy()
    v_ctx = np.asarray(v_cache)[cs].copy()
    order = np.argsort(sm, kind="stable")
    ss = sm[order]
    j = np.searchsorted(ss, cs)
    jc = np.minimum(j, len(ss) - 1)
    hit = ss[jc] == cs
    if hit.any():
        src = order[jc[hit]]
        k_ctx[hit] = kh[src]
        v_ctx[hit] = vh[src]

    k_all = np.concatenate([k_ctx, kh], axis=0)  # [L, NKVH, HD]
    v_all = np.concatenate([v_ctx, vh], axis=0)
    return k_all, v_all


# results of the last run (exec time etc), for the local test harness
last_results = None


def kernel(q, k, v, k_cache, v_cache, slot_mapping, context_slots):
    global last_results
    q = np.asarray(q, dtype=np.float32)
    k_all, v_all = _prep_host(
        q, np.asarray(k), np.asarray(v), k_cache, v_cache,
        slot_mapping, context_slots,
    )

    if "nc" not in _CACHE:
        _CACHE["nc"] = _build()
    nc = _CACHE["nc"]

    tri = np.where(
        np.arange(128)[None, :] >= np.arange(128)[:, None], 1.0, 0.0
    ).astype(np.float16)

    in_maps = []
    for d in range(NDEV):
        in_maps.append(
            {
                "qdT": np.ascontiguousarray(
                    q[:, d * HPD * HD : (d + 1) * HPD * HD].T
                ).astype(np.float16),
                # [NKC, HD, L//NKC]: contiguous per-chunk kT blocks
                "kdT": np.ascontiguousarray(
                    k_all[:, d, :]
                    .T.reshape(HD, 4, L // 4)
                    .transpose(1, 0, 2)
                ).astype(np.float16),
                # [NKC, 128, TPC, HD]: partition p holds v[tile*128+p, :]
                "vd": np.ascontiguousarray(
                    v_all[:, d, :]
                    .reshape(4, 8, 128, HD)
                    .transpose(0, 2, 1, 3)
                ).astype(np.float16),
                "tri": tri,
            }
        )

    res = run_bass_kernel_spmd(nc, in_maps, core_ids=list(range(NDEV)))
    last_results = res

    out = np.empty((SEQ, NH * HD), dtype=np.float32)
    for d in range(NDEV):
        odr = res.results[d]["od"].astype(np.float32)
        oT = odr.transpose(1, 2, 0, 3).reshape(HPD, HD, SEQ)
        sb = res.results[d]["sums"]  # [NQC*HPD//2, 4, QCH]
        sums = np.empty((HPD, SEQ), dtype=np.float32)
        for c in range(NQC):
            for hp in range(HPD // 2):
                blk = sb[c * (HPD // 2) + hp]
                for j in range(2):
                    sums[2 * hp + j, c * QCH : (c + 1) * QCH] = (
                        blk[j] + blk[2 + j]
                    )
        o = oT / sums[:, None, :]
        out[:, d * HPD * HD : (d + 1) * HPD * HD] = (
            o.transpose(2, 0, 1).reshape(SEQ, HPD * HD)
        )
    return out


# revision 23
# speedup vs baseline: 1.0247x; 1.0052x over previous
"""Chunked-prefill paged attention kernel for Trainium2 (Bass/Tile), 8 cores.

Sharding: tensor-parallel over heads. Core i handles q heads 4i..4i+3 and
kv head i. The paged-cache scatter/gather (index-driven data movement) is
resolved on the host; each core runs dense attention over the gathered
[ctx | chunk] keys/values for its kv head.

Per-core structure ("transposed scores"): loop over (q-chunk c, head-pair
hp); inner loop over 128-row l-tiles, software-pipelined one step so the
activation engine (the bottleneck at ~1.15 us per [128,2,512] exp) never
starves:
  - 2 QK^T matmuls (fp16, kv-head kT stationary shared by both heads,
    LDWEIGHTS fully hidden behind the streams) -> fp32 PSUM pair tile
    [128,2,512] (2 banks, double-buffered).
  - causal mask: DVE adds a NEG-triangle on the diagonal 128-block; QK/PV
    and the exp are exactly trimmed to the visible q-columns.
  - ONE activation exps both heads' scores -> fp16 ex tile in SBUF.
  - 2 PV matmuls (fp16) accumulate into per-head PSUM banks.
  - 2 col-tiled (tile_position) ones-matmuls run CONCURRENTLY on separate
    XBUSes, accumulating both heads' softmax denominators into rows
    {0,32} of ONE persistent PSUM bank across the whole pass - one
    512-col stream per tile instead of two.
PSUM: 4 (score pairs x2) + 2 (accumulators) + 1 (denominators) = 7 banks.
The unnormalized oT and denominators are DMA'd out; the host does the
final divide and [d, q] -> [q, d] transpose.
"""

import numpy as np

import concourse.bacc as bacc
import concourse.bass as bass
import concourse.mybir as mybir
import concourse.tile as tile
from concourse.bass_utils import run_bass_kernel_spmd

NH, NKVH, HD = 32, 8, 128
SCALE = 0.08838834764831845  # 1/sqrt(128)
SEQ, CTX = 1024, 3072
L = CTX + SEQ  # 4096
NDEV = 8
HPD = NH // NDEV  # q heads per device
QCH = 512  # q columns per chunk (psum bank width in f32)
NQC = SEQ // QCH
NT = L // 128  # 32 l-tiles
NT_CTX = CTX // 128  # 24 context l-tiles
NEG = -1.0e30

F32 = mybir.dt.float32
FP16 = mybir.dt.float16

_CACHE = {}


def _tiles_for_chunk(c):
    """(lt, st, diag) per l-tile: st = first visible q-col, diag = needs
    triangular mask at cols [st, st+128)."""
    out = [(lt, 0, False) for lt in range(NT_CTX)]
    for b in range(4 * (c + 1)):
        st = 128 * b - QCH * c
        out.append((NT_CTX + b, max(st, 0), st >= 0))
    return out


def _build():
    nc = bacc.Bacc("TRN2", target_bir_lowering=False, debug=False)

    NKC = 4
    TPC = NT // NKC  # l-tiles per load chunk
    qdT = nc.dram_tensor("qdT", [HPD * HD, SEQ], FP16, kind="ExternalInput")
    kdT = nc.dram_tensor(
        "kdT", [NKC, HD, L // NKC], FP16, kind="ExternalInput"
    )
    vd = nc.dram_tensor(
        "vd", [NKC, HD, TPC, HD], FP16, kind="ExternalInput"
    )
    tri = nc.dram_tensor("tri", [128, 128], FP16, kind="ExternalInput")
    # lead-in blob: [kT chunk 0 | qT head 0 | qT head 1] in one DMA
    lead = nc.dram_tensor(
        "lead", [HD, 3, L // 4], FP16, kind="ExternalInput"
    )
    od = nc.dram_tensor(
        "od", [NQC, HPD, HD, QCH], FP16, kind="ExternalOutput"
    )
    sums_out = nc.dram_tensor(
        "sums", [NQC * HPD // 2, 4, QCH], F32, kind="ExternalOutput"
    )

    with tile.TileContext(nc) as tc:
        with (
            tc.tile_pool(name="inp", bufs=1) as inp,
            tc.tile_pool(name="small", bufs=1) as small,
            tc.tile_pool(name="exq", bufs=10) as exq,
            tc.tile_pool(name="ssb", bufs=2) as ssb,
            tc.tile_pool(name="osb", bufs=4) as osb,
            tc.tile_pool(name="scp", bufs=2, space="PSUM") as scp,
            tc.tile_pool(name="accps", bufs=1, space="PSUM") as accps,
            tc.tile_pool(name="sumps", bufs=1, space="PSUM") as sumps,
        ):
            # ---- constants ----
            tri_sb = small.tile([128, 128], FP16, tag="tri")
            nc.scalar.dma_start(out=tri_sb, in_=tri[:, :])
            ones_f = small.tile([128, 1], F32, tag="ones_f")
            nc.vector.memset(ones_f, 1.0)
            ones_h = small.tile([128, 1], FP16, tag="ones")
            nc.vector.tensor_copy(out=ones_h, in_=ones_f)

            # ---- input loads: all-contiguous fp16 DMA chunks.
            # DMA triggers cost ~650ns on the issuing engine: keep few. ----
            lead_sb = inp.tile(
                [128, 3, L // 4], FP16, name="lead_sb", tag="lead_sb"
            )
            kT = [None] + [
                inp.tile([128, L // NKC], FP16, name=f"kT{i}", tag=f"kT{i}")
                for i in range(1, NKC)
            ]
            qT = [lead_sb[:, 1, :], lead_sb[:, 2, :]] + [
                inp.tile([128, SEQ], FP16, name=f"qT{h}", tag=f"qT{h}")
                for h in range(2, HPD)
            ]
            v_h = [
                inp.tile([128, TPC, HD], FP16, name=f"v{i}", tag=f"v{i}")
                for i in range(NKC)
            ]

            def load_v(i):
                nc.scalar.dma_start(out=v_h[i], in_=vd[i, :, :, :])

            nc.sync.dma_start(out=lead_sb, in_=lead[:, :, :])
            load_v(0)
            for i in range(1, NKC):
                nc.sync.dma_start(out=kT[i], in_=kdT[i, :, :])
                load_v(i)
            for h in range(2, HPD):
                nc.sync.dma_start(
                    out=qT[h], in_=qdT[h * 128 : (h + 1) * 128, :]
                )

            def kT_at(lt):
                i, o = lt // TPC, (lt % TPC) * 128
                if i == 0:
                    return lead_sb[:, 0, o : o + 128]
                return kT[i][:, o : o + 128]

            def v_at(lt):
                return v_h[lt // TPC][:, lt % TPC, :]

            assert TPC == 8  # kT_at/v_at index by lt // TPC

            # ---- main: one flat software pipeline across all 4 passes
            # (q-chunk c x head-pair hp), so the pipeline never drains at
            # pass boundaries: pass p's PV/sums tails interleave with pass
            # p+1's QK/exp ramp. ----
            passes = [
                (c, hp, _tiles_for_chunk(c))
                for c in range(NQC)
                for hp in range(HPD // 2)
            ]
            flat = [
                (p, i)
                for p, (_, _, tiles) in enumerate(passes)
                for i in range(len(tiles))
            ]
            ctx = {}  # pass idx -> dict(acc, sums_ps, ex_tiles)

            def emit_qk(p, i):
                c, hp, tiles = passes[p]
                if i == 0:
                    ctx[p] = {
                        "acc": [
                            accps.tile(
                                [128, QCH], F32, name=f"acc{j}", tag=f"acc{j}"
                            )
                            for j in range(2)
                        ],
                        "sums_ps": sumps.tile(
                            [97, QCH], F32, name="sums_ps", tag="sums_ps"
                        ),
                        "ex": [None] * len(tiles),
                    }
                lt, st, diag = tiles[i]
                qsl = slice(c * QCH + st, (c + 1) * QCH)
                pair = scp.tile([128, 2, QCH], F32, name="pair", tag="pair")
                for j in range(2):
                    nc.tensor.matmul(
                        pair[:, j, st:],
                        kT_at(lt),
                        qT[2 * hp + j][:, qsl],
                        start=True,
                        stop=True,
                    )
                exi = exq.tile([128, 2, QCH], FP16, name="exi", tag="ex")
                nc.scalar.activation(
                    out=exi[:, :, st:],
                    in_=pair[:, :, st:],
                    func=mybir.ActivationFunctionType.Exp,
                    scale=SCALE,
                )
                if diag:
                    for j in range(2):
                        nc.vector.tensor_mul(
                            out=exi[:, j, st : st + 128],
                            in0=exi[:, j, st : st + 128],
                            in1=tri_sb,
                        )
                ctx[p]["ex"][i] = exi

            def emit_sums(p, i0):
                # 8 denominator matmuls for 4 iterations in two 4-way
                # col-tiled concurrent waves; row = 64*(iter parity) +
                # 32*(head), accumulated across the pass in one PSUM bank.
                _, _, tiles = passes[p]
                cp = ctx[p]
                last_i = len(tiles) - 1
                for i in range(i0, i0 + 4):
                    lt, st, diag = tiles[i]
                    exi = cp["ex"][i]
                    for j in range(2):
                        r = 64 * (i % 2) + 32 * j
                        nc.tensor.matmul(
                            cp["sums_ps"][r : r + 1, st:],
                            ones_h,
                            exi[:, j, st:],
                            start=(i < 2),
                            stop=(i >= last_i - 1),
                            tile_position=(0, r),
                            skip_group_check=True,
                        )
                if i0 + 4 == len(tiles):
                    emit_drains(p)

            def emit_pv(p, ia):
                _, _, tiles = passes[p]
                cp = ctx[p]
                last_i = len(tiles) - 1
                for i in (ia, ia + 1):
                    lt, st, diag = tiles[i]
                    exi = cp["ex"][i]
                    for j in range(2):
                        nc.tensor.matmul(
                            cp["acc"][j][:, st:],
                            v_at(lt),
                            exi[:, j, st:],
                            start=(i == 0),
                            stop=(i == last_i),
                            skip_group_check=True,
                        )

            def emit_drains(p):
                c, hp, tiles = passes[p]
                cp = ctx[p]
                pidx = c * (HPD // 2) + hp
                sums_sb = ssb.tile([97, QCH], F32, tag="sums_sb")
                nc.vector.tensor_copy(out=sums_sb, in_=cp["sums_ps"])
                nc.sync.dma_start(
                    out=sums_out[pidx, :, :], in_=sums_sb[0:97:32, :]
                )
                for j in range(2):
                    acc_sb = osb.tile([128, QCH], FP16, tag="acc_sb")
                    if j == 0:
                        nc.vector.tensor_copy(out=acc_sb, in_=cp["acc"][j])
                    else:
                        nc.scalar.copy(out=acc_sb, in_=cp["acc"][j])
                    nc.sync.dma_start(out=od[c, 2 * hp + j, :, :], in_=acc_sb)

            # pipeline: QK at w, PV lags 4 iterations, sums lag 8
            for w in range(0, len(flat) + 8, 2):
                if w < len(flat):
                    emit_qk(*flat[w])
                    emit_qk(*flat[w + 1])
                if w >= 8 and (w - 8) % 4 == 0 and w - 8 < len(flat):
                    emit_sums(*flat[w - 8])
                if w >= 4 and w - 4 < len(flat):
                    emit_pv(*flat[w - 4])
            # note: emit_sums at i0+4==n triggers the pass's drains; the
            # (w-8)%4 alignment holds because every pass length %4 == 0
    nc.compile()
    return nc


def _prep_host(q, k, v, k_cache, v_cache, slot_mapping, context_slots):
    """Resolve the paged-cache scatter+gather on the host."""
    kh = np.ascontiguousarray(k).reshape(SEQ, NKVH, HD)
    vh = np.ascontiguousarray(v).reshape(SEQ, NKVH, HD)
    sm = np.asarray(slot_mapping)
    cs = np.asarray(context_slots)

    k_ctx = np.asarray(k_cache)[cs].copy()
    v_ctx = np.asarray(v_cache)[cs].copy()
    order = np.argsort(sm, kind="stable")
    ss = sm[order]
    j = np.searchsorted(ss, cs)
    jc = np.minimum(j, len(ss) - 1)
    hit = ss[jc] == cs
    if hit.any():
        src = order[jc[hit]]
        k_ctx[hit] = kh[src]
        v_ctx[hit] = vh[src]

    k_all = np.concatenate([k_ctx, kh], axis=0)  # [L, NKVH, HD]
    v_all = np.concatenate([v_ctx, vh], axis=0)
    return k_all, v_all


# results of the last run (exec time etc), for the local test harness
last_results = None


def kernel(q, k, v, k_cache, v_cache, slot_mapping, context_slots):
    global last_results
    q = np.asarray(q, dtype=np.float32)
    k_all, v_all = _prep_host(
        q, np.asarray(k), np.asarray(v), k_cache, v_cache,
        slot_mapping, context_slots,
    )

    if "nc" not in _CACHE:
        _CACHE["nc"] = _build()
    nc = _CACHE["nc"]

    tri = np.where(
        np.arange(128)[None, :] >= np.arange(128)[:, None], 1.0, 0.0
    ).astype(np.float16)

    in_maps = []
    for d in range(NDEV):
        in_maps.append(
            {
                "qdT": np.ascontiguousarray(
                    q[:, d * HPD * HD : (d + 1) * HPD * HD].T
                ).astype(np.float16),
                "lead": np.ascontiguousarray(
                    np.stack(
                        [
                            k_all[0 : L // 4, d, :].T,
                            q[:, (d * HPD + 0) * HD : (d * HPD + 1) * HD].T,
                            q[:, (d * HPD + 1) * HD : (d * HPD + 2) * HD].T,
                        ],
                        axis=1,
                    )
                ).astype(np.float16),
                # [NKC, HD, L//NKC]: contiguous per-chunk kT blocks
                "kdT": np.ascontiguousarray(
                    k_all[:, d, :]
                    .T.reshape(HD, 4, L // 4)
                    .transpose(1, 0, 2)
                ).astype(np.float16),
                # [NKC, 128, TPC, HD]: partition p holds v[tile*128+p, :]
                "vd": np.ascontiguousarray(
                    v_all[:, d, :]
                    .reshape(4, 8, 128, HD)
                    .transpose(0, 2, 1, 3)
                ).astype(np.float16),
                "tri": tri,
            }
        )

    res = run_bass_kernel_spmd(nc, in_maps, core_ids=list(range(NDEV)))
    last_results = res

    out = np.empty((SEQ, NH * HD), dtype=np.float32)
    for d in range(NDEV):
        odr = res.results[d]["od"].astype(np.float32)
        oT = odr.transpose(1, 2, 0, 3).reshape(HPD, HD, SEQ)
        sb = res.results[d]["sums"]  # [NQC*HPD//2, 4, QCH]
        sums = np.empty((HPD, SEQ), dtype=np.float32)
        for c in range(NQC):
            for hp in range(HPD // 2):
                blk = sb[c * (HPD // 2) + hp]
                for j in range(2):
                    sums[2 * hp + j, c * QCH : (c + 1) * QCH] = (
                        blk[j] + blk[2 + j]
                    )
        o = oT / sums[:, None, :]
        out[:, d * HPD * HD : (d + 1) * HPD * HD] = (
            o.transpose(2, 0, 1).reshape(SEQ, HPD * HD)
        )
    return out


# revision 24
# speedup vs baseline: 1.0326x; 1.0078x over previous
"""Chunked-prefill paged attention kernel for Trainium2 (Bass/Tile), 8 cores.

Sharding: tensor-parallel over heads. Core i handles q heads 4i..4i+3 and
kv head i. The paged-cache scatter/gather (index-driven data movement) is
resolved on the host; each core runs dense attention over the gathered
[ctx | chunk] keys/values for its kv head.

Per-core structure ("transposed scores"): loop over (q-chunk c, head-pair
hp); inner loop over 128-row l-tiles, software-pipelined one step so the
activation engine (the bottleneck at ~1.15 us per [128,2,512] exp) never
starves:
  - 2 QK^T matmuls (fp16, kv-head kT stationary shared by both heads,
    LDWEIGHTS fully hidden behind the streams) -> fp32 PSUM pair tile
    [128,2,512] (2 banks, double-buffered).
  - causal mask: DVE adds a NEG-triangle on the diagonal 128-block; QK/PV
    and the exp are exactly trimmed to the visible q-columns.
  - ONE activation exps both heads' scores -> fp16 ex tile in SBUF.
  - 2 PV matmuls (fp16) accumulate into per-head PSUM banks.
  - 2 col-tiled (tile_position) ones-matmuls run CONCURRENTLY on separate
    XBUSes, accumulating both heads' softmax denominators into rows
    {0,32} of ONE persistent PSUM bank across the whole pass - one
    512-col stream per tile instead of two.
PSUM: 4 (score pairs x2) + 2 (accumulators) + 1 (denominators) = 7 banks.
The unnormalized oT and denominators are DMA'd out; the host does the
final divide and [d, q] -> [q, d] transpose.
"""

import numpy as np

import concourse.bacc as bacc
import concourse.bass as bass
import concourse.mybir as mybir
import concourse.tile as tile
from concourse.bass_utils import run_bass_kernel_spmd

NH, NKVH, HD = 32, 8, 128
SCALE = 0.08838834764831845  # 1/sqrt(128)
SEQ, CTX = 1024, 3072
L = CTX + SEQ  # 4096
NDEV = 8
HPD = NH // NDEV  # q heads per device
QCH = 512  # q columns per chunk (psum bank width in f32)
NQC = SEQ // QCH
NT = L // 128  # 32 l-tiles
NT_CTX = CTX // 128  # 24 context l-tiles
NEG = -1.0e30

F32 = mybir.dt.float32
FP16 = mybir.dt.float16

_CACHE = {}


def _tiles_for_chunk(c):
    """(lt, st, diag) per l-tile: st = first visible q-col, diag = needs
    triangular mask at cols [st, st+128)."""
    out = [(lt, 0, False) for lt in range(NT_CTX)]
    for b in range(4 * (c + 1)):
        st = 128 * b - QCH * c
        out.append((NT_CTX + b, max(st, 0), st >= 0))
    return out


def _build():
    nc = bacc.Bacc("TRN2", target_bir_lowering=False, debug=False)

    NKC = 4
    TPC = NT // NKC  # l-tiles per load chunk
    qdT = nc.dram_tensor("qdT", [HPD * HD, SEQ], FP16, kind="ExternalInput")
    kdT = nc.dram_tensor(
        "kdT", [NKC, HD, L // NKC], FP16, kind="ExternalInput"
    )
    vd = nc.dram_tensor(
        "vd", [NKC, HD, TPC, HD], FP16, kind="ExternalInput"
    )
    tri = nc.dram_tensor("tri", [128, 128], FP16, kind="ExternalInput")
    # micro-lead blob gating the first matmuls: [kT l-tiles 0-3 |
    # qT head 0 cols 0-511 | qT head 1 cols 0-511] in one DMA
    lead = nc.dram_tensor(
        "lead", [HD, 3, QCH], FP16, kind="ExternalInput"
    )
    od = nc.dram_tensor(
        "od", [NQC, HPD, HD, QCH], FP16, kind="ExternalOutput"
    )
    sums_out = nc.dram_tensor(
        "sums", [NQC * HPD // 2, 4, QCH], F32, kind="ExternalOutput"
    )

    with tile.TileContext(nc) as tc:
        with (
            tc.tile_pool(name="inp", bufs=1) as inp,
            tc.tile_pool(name="small", bufs=1) as small,
            tc.tile_pool(name="exq", bufs=10) as exq,
            tc.tile_pool(name="ssb", bufs=2) as ssb,
            tc.tile_pool(name="osb", bufs=4) as osb,
            tc.tile_pool(name="scp", bufs=2, space="PSUM") as scp,
            tc.tile_pool(name="accps", bufs=1, space="PSUM") as accps,
            tc.tile_pool(name="sumps", bufs=1, space="PSUM") as sumps,
        ):
            # ---- constants ----
            tri_sb = small.tile([128, 128], FP16, tag="tri")
            nc.scalar.dma_start(out=tri_sb, in_=tri[:, :])
            ones_f = small.tile([128, 1], F32, tag="ones_f")
            nc.vector.memset(ones_f, 1.0)
            ones_h = small.tile([128, 1], FP16, tag="ones")
            nc.vector.tensor_copy(out=ones_h, in_=ones_f)

            # ---- input loads: all-contiguous fp16 DMA chunks.
            # DMA triggers cost ~650ns on the issuing engine: keep few. ----
            lead_sb = inp.tile(
                [128, 3, QCH], FP16, name="lead_sb", tag="lead_sb"
            )
            kT = [
                inp.tile([128, L // NKC], FP16, name=f"kT{i}", tag=f"kT{i}")
                for i in range(NKC)
            ]
            qT = [
                inp.tile([128, SEQ], FP16, name=f"qT{h}", tag=f"qT{h}")
                for h in range(HPD)
            ]
            v_h = [
                inp.tile([128, TPC, HD], FP16, name=f"v{i}", tag=f"v{i}")
                for i in range(NKC)
            ]

            def load_v(i):
                nc.scalar.dma_start(out=v_h[i], in_=vd[i, :, :, :])

            nc.sync.dma_start(out=lead_sb, in_=lead[:, :, :])
            nc.sync.dma_start(out=kT[0], in_=kdT[0, :, :])
            load_v(0)
            nc.sync.dma_start(out=qT[0], in_=qdT[0:128, :])
            nc.sync.dma_start(out=qT[1], in_=qdT[128:256, :])
            for i in range(1, NKC):
                nc.sync.dma_start(out=kT[i], in_=kdT[i, :, :])
                load_v(i)
            for h in range(2, HPD):
                nc.sync.dma_start(
                    out=qT[h], in_=qdT[h * 128 : (h + 1) * 128, :]
                )

            def kT_at(lt):
                if lt < 4:
                    return lead_sb[:, 0, lt * 128 : (lt + 1) * 128]
                i, o = lt // TPC, (lt % TPC) * 128
                return kT[i][:, o : o + 128]

            def v_at(lt):
                return v_h[lt // TPC][:, lt % TPC, :]

            assert TPC == 8  # kT_at/v_at index by lt // TPC

            # ---- main: one flat software pipeline across all 4 passes
            # (q-chunk c x head-pair hp), so the pipeline never drains at
            # pass boundaries: pass p's PV/sums tails interleave with pass
            # p+1's QK/exp ramp. ----
            passes = [
                (c, hp, _tiles_for_chunk(c))
                for c in range(NQC)
                for hp in range(HPD // 2)
            ]
            flat = [
                (p, i)
                for p, (_, _, tiles) in enumerate(passes)
                for i in range(len(tiles))
            ]
            ctx = {}  # pass idx -> dict(acc, sums_ps, ex_tiles)

            def emit_qk(p, i):
                c, hp, tiles = passes[p]
                if i == 0:
                    ctx[p] = {
                        "acc": [
                            accps.tile(
                                [128, QCH], F32, name=f"acc{j}", tag=f"acc{j}"
                            )
                            for j in range(2)
                        ],
                        "sums_ps": sumps.tile(
                            [97, QCH], F32, name="sums_ps", tag="sums_ps"
                        ),
                        "ex": [None] * len(tiles),
                    }
                lt, st, diag = tiles[i]
                pair = scp.tile([128, 2, QCH], F32, name="pair", tag="pair")
                for j in range(2):
                    h = 2 * hp + j
                    if c == 0 and h < 2:
                        qmv = lead_sb[:, 1 + h, st:]
                    else:
                        qmv = qT[h][:, c * QCH + st : (c + 1) * QCH]
                    nc.tensor.matmul(
                        pair[:, j, st:],
                        kT_at(lt),
                        qmv,
                        start=True,
                        stop=True,
                    )
                exi = exq.tile([128, 2, QCH], FP16, name="exi", tag="ex")
                nc.scalar.activation(
                    out=exi[:, :, st:],
                    in_=pair[:, :, st:],
                    func=mybir.ActivationFunctionType.Exp,
                    scale=SCALE,
                )
                if diag:
                    for j in range(2):
                        nc.vector.tensor_mul(
                            out=exi[:, j, st : st + 128],
                            in0=exi[:, j, st : st + 128],
                            in1=tri_sb,
                        )
                ctx[p]["ex"][i] = exi

            def emit_sums(p, i0):
                # 8 denominator matmuls for 4 iterations in two 4-way
                # col-tiled concurrent waves; row = 64*(iter parity) +
                # 32*(head), accumulated across the pass in one PSUM bank.
                _, _, tiles = passes[p]
                cp = ctx[p]
                last_i = len(tiles) - 1
                for i in range(i0, i0 + 4):
                    lt, st, diag = tiles[i]
                    exi = cp["ex"][i]
                    for j in range(2):
                        r = 64 * (i % 2) + 32 * j
                        nc.tensor.matmul(
                            cp["sums_ps"][r : r + 1, st:],
                            ones_h,
                            exi[:, j, st:],
                            start=(i < 2),
                            stop=(i >= last_i - 1),
                            tile_position=(0, r),
                            skip_group_check=True,
                        )
                if i0 + 4 == len(tiles):
                    emit_drains(p)

            def emit_pv(p, ia):
                _, _, tiles = passes[p]
                cp = ctx[p]
                last_i = len(tiles) - 1
                for i in (ia, ia + 1):
                    lt, st, diag = tiles[i]
                    exi = cp["ex"][i]
                    for j in range(2):
                        nc.tensor.matmul(
                            cp["acc"][j][:, st:],
                            v_at(lt),
                            exi[:, j, st:],
                            start=(i == 0),
                            stop=(i == last_i),
                            skip_group_check=True,
                        )

            def emit_drains(p):
                c, hp, tiles = passes[p]
                cp = ctx[p]
                pidx = c * (HPD // 2) + hp
                sums_sb = ssb.tile([97, QCH], F32, tag="sums_sb")
                nc.vector.tensor_copy(out=sums_sb, in_=cp["sums_ps"])
                nc.sync.dma_start(
                    out=sums_out[pidx, :, :], in_=sums_sb[0:97:32, :]
                )
                for j in range(2):
                    acc_sb = osb.tile([128, QCH], FP16, tag="acc_sb")
                    if j == 0:
                        nc.vector.tensor_copy(out=acc_sb, in_=cp["acc"][j])
                    else:
                        nc.scalar.copy(out=acc_sb, in_=cp["acc"][j])
                    nc.sync.dma_start(out=od[c, 2 * hp + j, :, :], in_=acc_sb)

            # pipeline: QK at w, PV lags 4 iterations, sums lag 8
            for w in range(0, len(flat) + 8, 2):
                if w < len(flat):
                    emit_qk(*flat[w])
                    emit_qk(*flat[w + 1])
                if w >= 8 and (w - 8) % 4 == 0 and w - 8 < len(flat):
                    emit_sums(*flat[w - 8])
                if w >= 4 and w - 4 < len(flat):
                    emit_pv(*flat[w - 4])
            # note: emit_sums at i0+4==n triggers the pass's drains; the
            # (w-8)%4 alignment holds because every pass length %4 == 0
    nc.compile()
    return nc


def _prep_host(q, k, v, k_cache, v_cache, slot_mapping, context_slots):
    """Resolve the paged-cache scatter+gather on the host."""
    kh = np.ascontiguousarray(k).reshape(SEQ, NKVH, HD)
    vh = np.ascontiguousarray(v).reshape(SEQ, NKVH, HD)
    sm = np.asarray(slot_mapping)
    cs = np.asarray(context_slots)

    k_ctx = np.asarray(k_cache)[cs].copy()
    v_ctx = np.asarray(v_cache)[cs].copy()
    order = np.argsort(sm, kind="stable")
    ss = sm[order]
    j = np.searchsorted(ss, cs)
    jc = np.minimum(j, len(ss) - 1)
    hit = ss[jc] == cs
    if hit.any():
        src = order[jc[hit]]
        k_ctx[hit] = kh[src]
        v_ctx[hit] = vh[src]

    k_all = np.concatenate([k_ctx, kh], axis=0)  # [L, NKVH, HD]
    v_all = np.concatenate([v_ctx, vh], axis=0)
    return k_all, v_all


# results of the last run (exec time etc), for the local test harness
last_results = None


def kernel(q, k, v, k_cache, v_cache, slot_mapping, context_slots):
    global last_results
    q = np.asarray(q, dtype=np.float32)
    k_all, v_all = _prep_host(
        q, np.asarray(k), np.asarray(v), k_cache, v_cache,
        slot_mapping, context_slots,
    )

    if "nc" not in _CACHE:
        _CACHE["nc"] = _build()
    nc = _CACHE["nc"]

    tri = np.where(
        np.arange(128)[None, :] >= np.arange(128)[:, None], 1.0, 0.0
    ).astype(np.float16)

    in_maps = []
    for d in range(NDEV):
        in_maps.append(
            {
                "qdT": np.ascontiguousarray(
                    q[:, d * HPD * HD : (d + 1) * HPD * HD].T
                ).astype(np.float16),
                "lead": np.ascontiguousarray(
                    np.stack(
                        [
                            k_all[0:QCH, d, :].T,
                            q[0:QCH, (d * HPD + 0) * HD : (d * HPD + 1) * HD].T,
                            q[0:QCH, (d * HPD + 1) * HD : (d * HPD + 2) * HD].T,
                        ],
                        axis=1,
                    )
                ).astype(np.float16),
                # [NKC, HD, L//NKC]: contiguous per-chunk kT blocks
                "kdT": np.ascontiguousarray(
                    k_all[:, d, :]
                    .T.reshape(HD, 4, L // 4)
                    .transpose(1, 0, 2)
                ).astype(np.float16),
                # [NKC, 128, TPC, HD]: partition p holds v[tile*128+p, :]
                "vd": np.ascontiguousarray(
                    v_all[:, d, :]
                    .reshape(4, 8, 128, HD)
                    .transpose(0, 2, 1, 3)
                ).astype(np.float16),
                "tri": tri,
            }
        )

    res = run_bass_kernel_spmd(nc, in_maps, core_ids=list(range(NDEV)))
    last_results = res

    out = np.empty((SEQ, NH * HD), dtype=np.float32)
    for d in range(NDEV):
        odr = res.results[d]["od"].astype(np.float32)
        oT = odr.transpose(1, 2, 0, 3).reshape(HPD, HD, SEQ)
        sb = res.results[d]["sums"]  # [NQC*HPD//2, 4, QCH]
        sums = np.empty((HPD, SEQ), dtype=np.float32)
        for c in range(NQC):
            for hp in range(HPD // 2):
                blk = sb[c * (HPD // 2) + hp]
                for j in range(2):
                    sums[2 * hp + j, c * QCH : (c + 1) * QCH] = (
                        blk[j] + blk[2 + j]
                    )
        o = oT / sums[:, None, :]
        out[:, d * HPD * HD : (d + 1) * HPD * HD] = (
            o.transpose(2, 0, 1).reshape(SEQ, HPD * HD)
        )
    return out


# revision 25
# speedup vs baseline: 1.0476x; 1.0145x over previous
"""Chunked-prefill paged attention kernel for Trainium2 (Bass/Tile), 8 cores.

Sharding: tensor-parallel over heads. Core i handles q heads 4i..4i+3 and
kv head i. The paged-cache scatter/gather (index-driven data movement) is
resolved on the host; each core runs dense attention over the gathered
[ctx | chunk] keys/values for its kv head.

Per-core structure ("transposed scores"): loop over (q-chunk c, head-pair
hp); inner loop over 128-row l-tiles, software-pipelined one step so the
activation engine (the bottleneck at ~1.15 us per [128,2,512] exp) never
starves:
  - 2 QK^T matmuls (fp16, kv-head kT stationary shared by both heads,
    LDWEIGHTS fully hidden behind the streams) -> fp32 PSUM pair tile
    [128,2,512] (2 banks, double-buffered).
  - causal mask: DVE adds a NEG-triangle on the diagonal 128-block; QK/PV
    and the exp are exactly trimmed to the visible q-columns.
  - ONE activation exps both heads' scores -> fp16 ex tile in SBUF.
  - 2 PV matmuls (fp16) accumulate into per-head PSUM banks.
  - 2 col-tiled (tile_position) ones-matmuls run CONCURRENTLY on separate
    XBUSes, accumulating both heads' softmax denominators into rows
    {0,32} of ONE persistent PSUM bank across the whole pass - one
    512-col stream per tile instead of two.
PSUM: 4 (score pairs x2) + 2 (accumulators) + 1 (denominators) = 7 banks.
The unnormalized oT and denominators are DMA'd out; the host does the
final divide and [d, q] -> [q, d] transpose.
"""

import numpy as np

import concourse.bacc as bacc
import concourse.bass as bass
import concourse.mybir as mybir
import concourse.tile as tile
from concourse.bass_utils import run_bass_kernel_spmd

NH, NKVH, HD = 32, 8, 128
SCALE = 0.08838834764831845  # 1/sqrt(128)
SEQ, CTX = 1024, 3072
L = CTX + SEQ  # 4096
NDEV = 8
HPD = NH // NDEV  # q heads per device
QCH = 512  # q columns per chunk (psum bank width in f32)
NQC = SEQ // QCH
NT = L // 128  # 32 l-tiles
NT_CTX = CTX // 128  # 24 context l-tiles
NEG = -1.0e30

F32 = mybir.dt.float32
FP16 = mybir.dt.float16

_CACHE = {}


def _tiles_for_chunk(c):
    """(lt, st, diag) per l-tile: st = first visible q-col, diag = needs
    triangular mask at cols [st, st+128)."""
    out = [(lt, 0, False) for lt in range(NT_CTX)]
    for b in range(4 * (c + 1)):
        st = 128 * b - QCH * c
        out.append((NT_CTX + b, max(st, 0), st >= 0))
    return out


def _build():
    nc = bacc.Bacc("TRN2", target_bir_lowering=False, debug=False)

    NKC = 4
    TPC = NT // NKC  # l-tiles per load chunk
    qdT = nc.dram_tensor("qdT", [HPD * HD, SEQ], FP16, kind="ExternalInput")
    kdT = nc.dram_tensor(
        "kdT", [NKC, HD, L // NKC], FP16, kind="ExternalInput"
    )
    vd = nc.dram_tensor(
        "vd", [NKC, HD, TPC, HD], FP16, kind="ExternalInput"
    )
    tri = nc.dram_tensor("tri", [128, 128], FP16, kind="ExternalInput")
    # micro-lead blob gating the first matmuls: [kT l-tiles 0-3 |
    # qT head 0 cols 0-511 | qT head 1 cols 0-511] in one DMA
    lead = nc.dram_tensor(
        "lead", [HD, 3, QCH], FP16, kind="ExternalInput"
    )
    od = nc.dram_tensor(
        "od", [NQC, HPD, HD, QCH], FP16, kind="ExternalOutput"
    )
    sums_out = nc.dram_tensor(
        "sums", [NQC * HPD // 2, 4, QCH], F32, kind="ExternalOutput"
    )

    with tile.TileContext(nc) as tc:
        with (
            tc.tile_pool(name="inp", bufs=1) as inp,
            tc.tile_pool(name="small", bufs=1) as small,
            tc.tile_pool(name="exq", bufs=10) as exq,
            tc.tile_pool(name="ssb", bufs=2) as ssb,
            tc.tile_pool(name="osb", bufs=4) as osb,
            tc.tile_pool(name="scp", bufs=2, space="PSUM") as scp,
            tc.tile_pool(name="accps", bufs=1, space="PSUM") as accps,
            tc.tile_pool(name="sumps", bufs=1, space="PSUM") as sumps,
        ):
            # ---- constants ----
            tri_sb = small.tile([128, 128], FP16, tag="tri")
            nc.scalar.dma_start(out=tri_sb, in_=tri[:, :])
            ones_f = small.tile([128, 1], F32, tag="ones_f")
            nc.vector.memset(ones_f, 1.0)
            ones_h = small.tile([128, 1], FP16, tag="ones")
            nc.vector.tensor_copy(out=ones_h, in_=ones_f)

            # ---- input loads: all-contiguous fp16 DMA chunks.
            # DMA triggers cost ~650ns on the issuing engine: keep few. ----
            lead_sb = inp.tile(
                [128, 3, QCH], FP16, name="lead_sb", tag="lead_sb"
            )
            kT = [
                inp.tile([128, L // NKC], FP16, name=f"kT{i}", tag=f"kT{i}")
                for i in range(NKC)
            ]
            qT = [
                inp.tile([128, SEQ], FP16, name=f"qT{h}", tag=f"qT{h}")
                for h in range(HPD)
            ]
            v_h = [
                inp.tile([128, TPC, HD], FP16, name=f"v{i}", tag=f"v{i}")
                for i in range(NKC)
            ]

            def load_v(i):
                nc.scalar.dma_start(out=v_h[i], in_=vd[i, :, :, :])

            nc.sync.dma_start(out=lead_sb, in_=lead[:, :, :])
            nc.sync.dma_start(out=kT[0], in_=kdT[0, :, :])
            load_v(0)
            nc.sync.dma_start(out=qT[0], in_=qdT[0:128, :])
            nc.sync.dma_start(out=qT[1], in_=qdT[128:256, :])
            for i in range(1, NKC):
                nc.sync.dma_start(out=kT[i], in_=kdT[i, :, :])
                load_v(i)
            for h in range(2, HPD):
                nc.sync.dma_start(
                    out=qT[h], in_=qdT[h * 128 : (h + 1) * 128, :]
                )

            def kT_at(lt):
                if lt < 4:
                    return lead_sb[:, 0, lt * 128 : (lt + 1) * 128]
                i, o = lt // TPC, (lt % TPC) * 128
                return kT[i][:, o : o + 128]

            def v_at(lt):
                return v_h[lt // TPC][:, lt % TPC, :]

            assert TPC == 8  # kT_at/v_at index by lt // TPC

            # ---- main: one flat software pipeline across all 4 passes
            # (q-chunk c x head-pair hp), so the pipeline never drains at
            # pass boundaries: pass p's PV/sums tails interleave with pass
            # p+1's QK/exp ramp. ----
            passes = [
                (c, hp, _tiles_for_chunk(c))
                for c in range(NQC)
                for hp in range(HPD // 2)
            ]
            flat = [
                (p, i)
                for p, (_, _, tiles) in enumerate(passes)
                for i in range(len(tiles))
            ]
            ctx = {}  # pass idx -> dict(acc, sums_ps, ex_tiles)

            def emit_qk(p, i):
                c, hp, tiles = passes[p]
                if i == 0:
                    ctx[p] = {
                        "acc": [
                            accps.tile(
                                [128, QCH], F32, name=f"acc{j}", tag=f"acc{j}"
                            )
                            for j in range(2)
                        ],
                        "sums_ps": sumps.tile(
                            [97, QCH], F32, name="sums_ps", tag="sums_ps"
                        ),
                        "ex": [None] * len(tiles),
                    }
                lt, st, diag = tiles[i]
                pair = scp.tile([128, 2, QCH], F32, name="pair", tag="pair")
                for j in range(2):
                    h = 2 * hp + j
                    if c == 0 and h < 2:
                        qmv = lead_sb[:, 1 + h, st:]
                    else:
                        qmv = qT[h][:, c * QCH + st : (c + 1) * QCH]
                    nc.tensor.matmul(
                        pair[:, j, st:],
                        kT_at(lt),
                        qmv,
                        start=True,
                        stop=True,
                    )
                exi = exq.tile([128, 2, QCH], FP16, name="exi", tag="ex")
                nc.scalar.activation(
                    out=exi[:, :, st:],
                    in_=pair[:, :, st:],
                    func=mybir.ActivationFunctionType.Exp,
                    scale=SCALE,
                )
                if diag:
                    for j in range(2):
                        nc.vector.tensor_mul(
                            out=exi[:, j, st : st + 128],
                            in0=exi[:, j, st : st + 128],
                            in1=tri_sb,
                        )
                ctx[p]["ex"][i] = exi

            def emit_sums(p, i0):
                # 8 denominator matmuls for 4 iterations in two 4-way
                # col-tiled concurrent waves; row = 64*(iter parity) +
                # 32*(head), accumulated across the pass in one PSUM bank.
                _, _, tiles = passes[p]
                cp = ctx[p]
                last_i = len(tiles) - 1
                for i in range(i0, i0 + 4):
                    lt, st, diag = tiles[i]
                    exi = cp["ex"][i]
                    for j in range(2):
                        r = 64 * (i % 2) + 32 * j
                        nc.tensor.matmul(
                            cp["sums_ps"][r : r + 1, st:],
                            ones_h,
                            exi[:, j, st:],
                            start=(i < 2),
                            stop=(i >= last_i - 1),
                            tile_position=(0, r),
                            skip_group_check=True,
                        )
                if i0 + 4 == len(tiles):
                    emit_drains(p)

            def emit_pv(p, ia):
                _, _, tiles = passes[p]
                cp = ctx[p]
                last_i = len(tiles) - 1
                for i in (ia, ia + 1):
                    lt, st, diag = tiles[i]
                    exi = cp["ex"][i]
                    for j in range(2):
                        nc.tensor.matmul(
                            cp["acc"][j][:, st:],
                            v_at(lt),
                            exi[:, j, st:],
                            start=(i == 0),
                            stop=(i == last_i),
                            skip_group_check=True,
                        )

            def emit_drains(p):
                c, hp, tiles = passes[p]
                cp = ctx[p]
                pidx = c * (HPD // 2) + hp
                sums_sb = ssb.tile([97, QCH], F32, tag="sums_sb")
                nc.scalar.copy(out=sums_sb, in_=cp["sums_ps"])
                nc.scalar.dma_start(
                    out=sums_out[pidx, :, :], in_=sums_sb[0:97:32, :]
                )
                for j in range(2):
                    acc_sb = osb.tile([128, QCH], FP16, tag="acc_sb")
                    if j == 0:
                        nc.vector.tensor_copy(out=acc_sb, in_=cp["acc"][j])
                    else:
                        nc.scalar.copy(out=acc_sb, in_=cp["acc"][j])
                    nc.sync.dma_start(out=od[c, 2 * hp + j, :, :], in_=acc_sb)

            # pipeline: QK at w, PV lags 4 iterations, sums lag 8
            for w in range(0, len(flat) + 8, 2):
                if w < len(flat):
                    emit_qk(*flat[w])
                    emit_qk(*flat[w + 1])
                if w >= 8 and (w - 8) % 4 == 0 and w - 8 < len(flat):
                    emit_sums(*flat[w - 8])
                if w >= 4 and w - 4 < len(flat):
                    emit_pv(*flat[w - 4])
            # note: emit_sums at i0+4==n triggers the pass's drains; the
            # (w-8)%4 alignment holds because every pass length %4 == 0
    nc.compile()
    return nc


def _prep_host(q, k, v, k_cache, v_cache, slot_mapping, context_slots):
    """Resolve the paged-cache scatter+gather on the host."""
    kh = np.ascontiguousarray(k).reshape(SEQ, NKVH, HD)
    vh = np.ascontiguousarray(v).reshape(SEQ, NKVH, HD)
    sm = np.asarray(slot_mapping)
    cs = np.asarray(context_slots)

    k_ctx = np.asarray(k_cache)[cs].copy()
    v_ctx = np.asarray(v_cache)[cs].copy()
    order = np.argsort(sm, kind="stable")
    ss = sm[order]
    j = np.searchsorted(ss, cs)
    jc = np.minimum(j, len(ss) - 1)
    hit = ss[jc] == cs
    if hit.any():
        src = order[jc[hit]]
        k_ctx[hit] = kh[src]
        v_ctx[hit] = vh[src]

    k_all = np.concatenate([k_ctx, kh], axis=0)  # [L, NKVH, HD]
    v_all = np.concatenate([v_ctx, vh], axis=0)
    return k_all, v_all


# results of the last run (exec time etc), for the local test harness
last_results = None


def kernel(q, k, v, k_cache, v_cache, slot_mapping, context_slots):
    global last_results
    q = np.asarray(q, dtype=np.float32)
    k_all, v_all = _prep_host(
        q, np.asarray(k), np.asarray(v), k_cache, v_cache,
        slot_mapping, context_slots,
    )

    if "nc" not in _CACHE:
        _CACHE["nc"] = _build()
    nc = _CACHE["nc"]

    tri = np.where(
        np.arange(128)[None, :] >= np.arange(128)[:, None], 1.0, 0.0
    ).astype(np.float16)

    in_maps = []
    for d in range(NDEV):
        in_maps.append(
            {
                "qdT": np.ascontiguousarray(
                    q[:, d * HPD * HD : (d + 1) * HPD * HD].T
                ).astype(np.float16),
                "lead": np.ascontiguousarray(
                    np.stack(
                        [
                            k_all[0:QCH, d, :].T,
                            q[0:QCH, (d * HPD + 0) * HD : (d * HPD + 1) * HD].T,
                            q[0:QCH, (d * HPD + 1) * HD : (d * HPD + 2) * HD].T,
                        ],
                        axis=1,
                    )
                ).astype(np.float16),
                # [NKC, HD, L//NKC]: contiguous per-chunk kT blocks
                "kdT": np.ascontiguousarray(
                    k_all[:, d, :]
                    .T.reshape(HD, 4, L // 4)
                    .transpose(1, 0, 2)
                ).astype(np.float16),
                # [NKC, 128, TPC, HD]: partition p holds v[tile*128+p, :]
                "vd": np.ascontiguousarray(
                    v_all[:, d, :]
                    .reshape(4, 8, 128, HD)
                    .transpose(0, 2, 1, 3)
                ).astype(np.float16),
                "tri": tri,
            }
        )

    res = run_bass_kernel_spmd(nc, in_maps, core_ids=list(range(NDEV)))
    last_results = res

    out = np.empty((SEQ, NH * HD), dtype=np.float32)
    for d in range(NDEV):
        odr = res.results[d]["od"].astype(np.float32)
        oT = odr.transpose(1, 2, 0, 3).reshape(HPD, HD, SEQ)
        sb = res.results[d]["sums"]  # [NQC*HPD//2, 4, QCH]
        sums = np.empty((HPD, SEQ), dtype=np.float32)
        for c in range(NQC):
            for hp in range(HPD // 2):
                blk = sb[c * (HPD // 2) + hp]
                for j in range(2):
                    sums[2 * hp + j, c * QCH : (c + 1) * QCH] = (
                        blk[j] + blk[2 + j]
                    )
        o = oT / sums[:, None, :]
        out[:, d * HPD * HD : (d + 1) * HPD * HD] = (
            o.transpose(2, 0, 1).reshape(SEQ, HPD * HD)
        )
    return out
